# revision 28
# baseline (speedup 1.0000x reference)
"""CovLoss (BCE + Dice + triple-Pearson) Trainium2 Bass kernel, v2.2.

Data parallel over batch: 32 samples -> 8 cores x 4 samples. Each core
streams fp16 logits/labels once, emits per-partition partial sums; host
combines in float64.

Engine plan (cost-model driven):
  - ACT: dummy Ln first (act-table load overlaps the first DMA), then
    Ln(p) / Ln(1-p)+accum per sample (engine floor ~15us), Square+accum
    for v2/h2, and two batched PSUM collapses (Copy+accum over bank rows;
    only rows 0/32/64 are meaningful, the rest is ignored garbage).
  - DVE: only ops with perf modes: tensor_scalar (4x) for masks/accums,
    tensor_tensor (2x) for the big products (D=lnp-lnq, y*D, y*M).
    scalar_tensor_tensor / tensor_tensor_reduce / custom DVE ops have NO
    perf modes (v1's mistake). Pool-finish kept on DVE (batched 2 samples
    per tensor_reduce over a 2-bank PSUM tile).
  - PE: row-pool matmuls (pooling), ones-reduces of y*D (16 matmuls into
    one accumulation row) and of y*M (per-sample rows at partition bases
    0/32/64 - the only legal matmul output bases).
  - GPSIMD can only memset/DMA (walrus rejects its tensor ops).
  - Queue discipline: each engine's emission order matches data readiness
    (in-order sequencers); poolfin after the products, l-moment tail
    interleaved into sample 3.
"""

import numpy as np

import concourse.bass as bass
import concourse.bacc as bacc
import concourse.tile as tile
from concourse import mybir
from concourse.bass_utils import run_bass_kernel_spmd

N_CORES = 8
N = 32
S_PER_CORE = N // N_CORES  # 4
H = W = 512
P = 128
T = H // P                 # 4 row blocks
FD = T * W                 # 2048 free elems per partition per sample
N2 = H // 4                # 128 pooled
K = N2 * N2
PPS = P // S_PER_CORE      # 32 partitions per sample in fat layout

F16 = mybir.dt.float16
F32 = mybir.dt.float32

# stats [128, 32] fp32 columns
C_M = 0      # cols 0..4: sum(M) (sample 0 split in two halves)
C_MYP = 5    # rows 0/32/64: sum(M*y) for samples 0..2 (PSUM collapse A)
C_BCP = 6    # row 32: sum(y*(lnp-lnq)) for samples 0..2 (PSUM collapse B)
C_BC3 = 7    # per-partition sum(y*D) sample 3 (DVE accum; PE is cold then)
C_MY3 = 8    # per-partition sum(M*y) sample 3 (DVE accum)
C_V, C_H, C_VH, C_V2, C_H2, C_VL, C_HL, C_VHL, C_L2, C_L = range(10, 20)
STATS_W = 32

ADD = mybir.AluOpType.add
MULT = mybir.AluOpType.mult
LN = mybir.ActivationFunctionType.Ln
SQUARE = mybir.ActivationFunctionType.Square
COPY = mybir.ActivationFunctionType.Copy


def _build_nc():
    nc = bacc.Bacc(trn_type="TRN2")

    d_logits = nc.dram_tensor("logits", [S_PER_CORE, P, FD], F16,
                              kind="ExternalInput")
    d_labels = nc.dram_tensor("labels", [S_PER_CORE, P, FD], F16,
                              kind="ExternalInput")
    d_att = nc.dram_tensor("att", [P, 2 * S_PER_CORE * N2], F16,
                           kind="ExternalInput")
    d_pool = nc.dram_tensor("poolmat", [P, T * P], F16, kind="ExternalInput")

    d_lpool = nc.dram_tensor("lpool", [S_PER_CORE, PPS, T, N2], F16,
                             kind="Internal")
    d_stats = nc.dram_tensor("stats", [P, STATS_W], F32,
                             kind="ExternalOutput")
    d_stats_act = nc.dram_tensor("stats_act", [P, S_PER_CORE + 1], F32,
                                 kind="ExternalOutput")

    with tile.TileContext(nc) as tc:
        with (
            tc.tile_pool(name="consts", bufs=1) as consts,
            tc.tile_pool(name="big", bufs=3) as big,
            tc.tile_pool(name="psum", bufs=2, space="PSUM") as psump,
            tc.tile_pool(name="psred", bufs=1, space="PSUM") as psred,
        ):
            stats = consts.tile([P, STATS_W], F32)
            stats_act = consts.tile([P, S_PER_CORE + 1], F32)
            attm = consts.tile([P, 2, S_PER_CORE * N2], F16)
            poolm = consts.tile([P, T, P], F16)
            lpool = consts.tile([P, S_PER_CORE, N2], F16)
            lfat = consts.tile([P, S_PER_CORE * N2], F16)
            vh = consts.tile([P, S_PER_CORE * N2], F16)
            ones = consts.tile([P, 1], F16)
            zeros65 = consts.tile([P, 65], F16)
            zerosw = consts.tile([P, W], F16)
            bias0 = consts.tile([P, 1], F16)
            bias1 = consts.tile([P, 1], F16)
            junkf = consts.tile([P, S_PER_CORE * N2], F16)
            junkv = consts.tile([P, S_PER_CORE * N2], F16)
            junkw = consts.tile([P, S_PER_CORE * N2], F16)
            cjunkA = consts.tile([65, W], F16)
            cjunkB = consts.tile([33, W], F16)

            vt = attm[:, 0, :]
            ht = attm[:, 1, :]

            nc.vector.memset(ones, 1.0)
            nc.gpsimd.memset(bias0, 0.0)
            nc.gpsimd.memset(bias1, 1.0)

            # dummy Ln: act-table load happens during the first input DMA
            nc.scalar.activation(out=bias1, in_=ones, func=LN, bias=bias0)
            nc.gpsimd.memset(bias1, 1.0)

            bankA = psred.tile([P, W], F32)   # rows 0/32/64: sum(M*y) s0..2
            bankB = psred.tile([P, W], F32)   # row 0: sum(M*y) s3; row 32: bce
            # zero the collapse windows so the Copy+accum reads no garbage
            nc.vector.memset(zeros65, 0.0)
            nc.gpsimd.memset(zerosw, 0.0)
            nc.tensor.matmul(bankA[0:65, :], lhsT=zeros65, rhs=zerosw,
                             start=True, stop=True, skip_group_check=True)
            nc.tensor.matmul(bankB[0:33, :], lhsT=zeros65[:, 0:33], rhs=zerosw,
                             start=True, stop=True, skip_group_check=True)
            # PE warmup: keep the tensor engine busy until the first pooling
            # matmuls so it reaches full pstate (cold PE runs 3.7x slower and
            # its latency cascades into the DVE pool-finish chain)
            for _ in range(14):
                nc.tensor.matmul(bankB[64:65, :], lhsT=zeros65[:, 0:1],
                                 rhs=zerosw, start=True, stop=True,
                                 skip_group_check=True)

            # input DMAs up front (SP in-order; p feeds ACT = critical)
            pts, yts = [], []
            for s in range(S_PER_CORE):
                p_tile = big.tile([P, FD], F16, tag=f"p{s}", name=f"p{s}")
                y_tile = big.tile([P, T, W], F16, tag=f"y{s}", name=f"y{s}")
                pts.append(p_tile)
                yts.append(y_tile)
            nc.sync.dma_start(out=pts[0][:, 0:FD // 2],
                              in_=d_logits[0][:, 0:FD // 2])
            nc.sync.dma_start(out=pts[0][:, FD // 2:FD],
                              in_=d_logits[0][:, FD // 2:FD])
            nc.sync.dma_start(
                out=yts[0], in_=d_labels[0].rearrange("p (t w) -> p t w", t=T))
            nc.sync.dma_start(
                out=attm, in_=d_att.rearrange("p (q f) -> p q f", q=2))
            nc.sync.dma_start(
                out=poolm, in_=d_pool.rearrange("p (t m) -> p t m", t=T))
            for s in (1, 2, 3):
                nc.sync.dma_start(out=pts[s], in_=d_logits[s])
                nc.sync.dma_start(
                    out=yts[s],
                    in_=d_labels[s].rearrange("p (t w) -> p t w", t=T))

            # PE: row-pool matmuls, interleaved with the reduces by readiness
            ps_pools = []
            for pair in range(2):
                ps_pools.append(psump.tile([P, 2 * W], F32, tag="pool",
                                           name=f"pspool{pair}"))

            def emit_pooling(s):
                half = ps_pools[s // 2][:, (s % 2) * W:(s % 2) * W + W]
                for t in range(T):
                    nc.tensor.matmul(
                        half, lhsT=poolm[:, t, :], rhs=yts[s][:, t, :],
                        start=(t == 0), stop=(t == T - 1),
                        skip_group_check=True)

            emit_pooling(0)
            emit_pooling(1)

            def poolfin(pair):
                with nc.allow_low_precision(reason="16-term pooled sums"):
                    nc.vector.tensor_reduce(
                        out=lpool.rearrange(
                            "p (u v) m -> p u v m", u=2)[:, pair],
                        in_=ps_pools[pair].rearrange(
                            "p (v g f) -> p v g f", v=2, f=4),
                        axis=mybir.AxisListType.X, op=ADD)

            for s in range(S_PER_CORE):
                pt, yt = pts[s], yts[s]
                ytf = yt.rearrange("p t w -> p (t w)")

                # mask: 4x tensor_scalar, accum -> sum(M) per sample
                # (sample 0 is split in halves to cut the startup latency;
                # the first half starts as soon as its DMA lands)
                dm = big.tile([P, 2, FD], F16, tag="dm")
                mt = dm[:, 1, :]
                if s == 0:
                    nc.vector.tensor_scalar(
                        out=mt[:, 0:FD // 2], in0=pt[:, 0:FD // 2],
                        scalar1=0.4, scalar2=None,
                        op0=mybir.AluOpType.is_gt, op1=ADD,
                        accum_out=stats[:, C_M:C_M + 1])
                    nc.vector.tensor_scalar(
                        out=mt[:, FD // 2:FD], in0=pt[:, FD // 2:FD],
                        scalar1=0.4, scalar2=None,
                        op0=mybir.AluOpType.is_gt, op1=ADD,
                        accum_out=stats[:, C_M + 1:C_M + 2])
                else:
                    nc.vector.tensor_scalar(
                        out=mt, in0=pt, scalar1=0.4, scalar2=None,
                        op0=mybir.AluOpType.is_gt, op1=ADD,
                        accum_out=stats[:, C_M + 1 + s:C_M + 2 + s])

                if s == 0:
                    # attention moments needing only v,h (fills DVE idle)
                    nc.vector.tensor_tensor(out=vh, in0=vt, in1=ht, op=MULT)
                    nc.vector.tensor_scalar(
                        out=junkf, in0=vh, scalar1=1.0, scalar2=None,
                        op0=MULT, op1=ADD, accum_out=stats[:, C_VH:C_VH + 1])
                    nc.vector.tensor_scalar(
                        out=junkf, in0=vt, scalar1=1.0, scalar2=None,
                        op0=MULT, op1=ADD, accum_out=stats[:, C_V:C_V + 1])
                    nc.vector.tensor_scalar(
                        out=junkf, in0=ht, scalar1=1.0, scalar2=None,
                        op0=MULT, op1=ADD, accum_out=stats[:, C_H:C_H + 1])

                # ACT: the two log passes (engine floor)
                lnp = big.tile([P, FD], F16, tag="lnp")
                lnq = big.tile([P, FD], F16, tag="lnq")
                if s == 0:
                    h = FD // 2
                    nc.scalar.activation(out=lnp[:, 0:h], in_=pt[:, 0:h],
                                         func=LN, bias=bias0)
                    nc.scalar.activation(
                        out=lnq[:, 0:h], in_=pt[:, 0:h], func=LN,
                        scale=-1.0, bias=bias1,
                        accum_out=stats_act[:, 0:1])
                    nc.scalar.activation(out=lnp[:, h:FD], in_=pt[:, h:FD],
                                         func=LN, bias=bias0)
                    nc.scalar.activation(
                        out=lnq[:, h:FD], in_=pt[:, h:FD], func=LN,
                        scale=-1.0, bias=bias1,
                        accum_out=stats_act[:, S_PER_CORE:S_PER_CORE + 1])
                else:
                    nc.scalar.activation(out=lnp, in_=pt, func=LN, bias=bias0)
                    nc.scalar.activation(
                        out=lnq, in_=pt, func=LN, scale=-1.0, bias=bias1,
                        accum_out=stats_act[:, s:s + 1])

                # DVE 2x products: D into dm[:,0,:], then ONE broadcast
                # tensor_tensor computes y*D and y*M together (y repeats via
                # a stride-0 middle dim; 2x mode only checks the last dim)
                nc.vector.tensor_tensor(
                    out=dm[:, 0, :], in0=lnp, in1=lnq,
                    op=mybir.AluOpType.subtract)
                if s == 2:
                    poolfin(1)
                    # bounce emitted before its readers (Tile deps follow
                    # emission order)
                    nc.sync.dma_start(
                        out=d_lpool.rearrange("s a b m -> (a b) s m"),
                        in_=lpool)
                    nc.sync.dma_start(
                        out=lfat,
                        in_=d_lpool.rearrange("s a b m -> (s a) (b m)"))
                pm = big.tile([P, 2, T, W], F16, tag="pm")
                ybc = ytf.rearrange("p (o f) -> p o f", o=1).broadcast_to(
                    [P, 2, FD])
                nc.vector.tensor_tensor(
                    out=pm.rearrange("p o t w -> p o (t w)"), in0=dm,
                    in1=ybc, op=MULT)
                yd = pm[:, 0]
                my = pm[:, 1]

                if s == 1:
                    poolfin(0)

                # PE: ones-reduces for samples 0..2 (sample 3 runs on DVE;
                # PE is cold by then). y*D accumulates into bankB row 32;
                # y*M per sample into bankA rows 0/32/64.
                if s < 3:
                    for c in range(T):
                        nc.tensor.matmul(
                            bankB[32:33, :], lhsT=ones, rhs=yd[:, c, :],
                            start=(s == 0 and c == 0),
                            stop=(s == 2 and c == T - 1),
                            skip_group_check=True)
                    for c in range(T):
                        nc.tensor.matmul(
                            bankA[PPS * s:PPS * s + 1, :], lhsT=ones,
                            rhs=my[:, c, :],
                            start=(c == 0), stop=(c == T - 1),
                            skip_group_check=True)
                if s < 2:
                    emit_pooling(s + 2)
                if s == S_PER_CORE - 1:
                    # PE is cold by now; reduce sample 3 on DVE instead
                    ydf3 = yd.rearrange("p t w -> p (t w)")
                    nc.vector.tensor_scalar(
                        out=ydf3, in0=ydf3,
                        scalar1=1.0, scalar2=None, op0=MULT, op1=ADD,
                        accum_out=stats[:, C_BC3:C_BC3 + 1])
                    myf3 = my.rearrange("p t w -> p (t w)")
                    nc.vector.tensor_scalar(
                        out=myf3, in0=myf3,
                        scalar1=1.0, scalar2=None, op0=MULT, op1=ADD,
                        accum_out=stats[:, C_MY3:C_MY3 + 1])

                if s == S_PER_CORE - 1:
                    # l-moment tail on DVE (lfat ready via early bounce)
                    nc.vector.tensor_tensor(
                        out=junkf, in0=vt, in1=lfat, op=MULT)
                    nc.vector.tensor_scalar(
                        out=junkf, in0=junkf, scalar1=1.0, scalar2=None,
                        op0=MULT, op1=ADD,
                        accum_out=stats[:, C_VL:C_VL + 1])
                    nc.vector.tensor_tensor(
                        out=junkf, in0=ht, in1=lfat, op=MULT)
                    nc.vector.tensor_scalar(
                        out=junkf, in0=junkf, scalar1=1.0, scalar2=None,
                        op0=MULT, op1=ADD,
                        accum_out=stats[:, C_HL:C_HL + 1])
                    nc.vector.tensor_tensor(
                        out=junkf, in0=vh, in1=lfat, op=MULT)
                    nc.vector.tensor_scalar(
                        out=junkf, in0=junkf, scalar1=1.0, scalar2=None,
                        op0=MULT, op1=ADD,
                        accum_out=stats[:, C_VHL:C_VHL + 1])

            # ACT tail: v,h second moments (own junk tiles - no WAR with DVE)
            nc.scalar.activation(
                out=junkv, in_=vt, func=SQUARE, bias=bias0,
                accum_out=stats[:, C_V2:C_V2 + 1])
            nc.scalar.activation(
                out=junkw, in_=ht, func=SQUARE, bias=bias0,
                accum_out=stats[:, C_H2:C_H2 + 1])

            # batched PSUM collapses on ACT: per-partition row sums; host
            # reads only rows 0/32/64 (A) and 32 (B)
            nc.scalar.activation(
                out=cjunkA, in_=bankA[0:65, :], func=COPY,
                accum_out=stats[0:65, C_MYP:C_MYP + 1])
            nc.scalar.activation(
                out=cjunkB[0:1, :], in_=bankB[32:33, :], func=COPY,
                accum_out=stats[32:33, C_BCP:C_BCP + 1])
            # l moments that need no product ride the idle ACT tail
            nc.scalar.activation(
                out=junkv, in_=lfat, func=SQUARE, bias=bias0,
                accum_out=stats[:, C_L2:C_L2 + 1])
            nc.scalar.activation(
                out=junkw, in_=lfat, func=COPY,
                accum_out=stats[:, C_L:C_L + 1])

            nc.sync.dma_start(out=d_stats[:, :], in_=stats)
            nc.sync.dma_start(out=d_stats_act[:, :], in_=stats_act)

    nc.compile()
    return nc


_NC_CACHE = None


def _get_nc():
    global _NC_CACHE
    if _NC_CACHE is None:
        _NC_CACHE = _build_nc()
    return _NC_CACHE


def _host_combine(stats_all, stats_act):
    """stats_all: [N_CORES, P, STATS_W] float64 -> scalar loss (float32)."""
    smooth = 1.0
    bce_sum = 0.0
    dice_sum = 0.0
    cor_sum = 0.0
    for i in range(N_CORES):
        st = stats_all[i]
        bce_sum += (st[32, C_BCP] + st[:, C_BC3].sum()
                    + stats_act[i].sum())
        for s in range(S_PER_CORE):
            my = st[PPS * s, C_MYP] if s < 3 else st[:, C_MY3].sum()
            if s == 0:
                m_cnt = st[:, C_M].sum() + st[:, C_M + 1].sum()
            else:
                m_cnt = st[:, C_M + 1 + s].sum()
            part = slice(PPS * s, PPS * (s + 1))
            sv = st[part, C_V].sum()
            sh = st[part, C_H].sum()
            svh = st[part, C_VH].sum()
            sv2 = st[part, C_V2].sum()
            sh2 = st[part, C_H2].sum()
            svl = st[part, C_VL].sum()
            shl = st[part, C_HL].sum()
            svhl = st[part, C_VHL].sum()
            sl2 = st[part, C_L2].sum()
            sl = st[part, C_L].sum()

            dice_sum += 2.0 * (my + smooth) / (m_cnt + sl + smooth)

            mv, mh, ml = sv / K, sh / K, sl / K
            num = svhl - mv * shl - mh * svl - ml * svh + 2.0 * K * mv * mh * ml
            den = np.sqrt((sv2 - K * mv * mv) * (sh2 - K * mh * mh)
                          * (sl2 - K * ml * ml))
            cor_sum += num / den

    bceloss = -bce_sum / (N * H * W)
    diceloss = 1.0 - dice_sum / N
    cor_loss = -cor_sum / N
    return np.float32(0.2 * bceloss + 0.3 * diceloss + 0.5 * cor_loss)


def _make_in_maps(logits, labels, v_attention, h_attention):
    f16 = np.float16

    # clamp AFTER fp16 rounding so Ln(1-p) never sees exactly 1.0
    pmax = np.float16(1.0 - 2.0 ** -11)
    lg = np.minimum(np.asarray(logits, np.float32).astype(f16), pmax)
    # square layout: row r = 128*t + p  ->  partition p, free t*512+w
    lg = np.ascontiguousarray(
        lg.reshape(N, T, P, W).transpose(0, 2, 1, 3).reshape(N, P, FD))
    lb = np.asarray(labels, np.float32).astype(f16)
    lb = np.ascontiguousarray(
        lb.reshape(N, T, P, W).transpose(0, 2, 1, 3).reshape(N, P, FD))

    # fat attention layout: partition 32*s + a holds rows [4a, 4a+4)
    va = np.asarray(v_attention, np.float32).astype(f16).reshape(N, N2, N2)
    ha = np.asarray(h_attention, np.float32).astype(f16).reshape(N, N2, N2)

    # poolm[p, t, m] = 1 iff m == 32*t + p//4 (row-pool chunk t)
    poolm = np.zeros((P, T, P), dtype=np.float32)
    for t in range(T):
        poolm[np.arange(P), t, 32 * t + np.arange(P) // 4] = 1.0
    poolm = poolm.reshape(P, T * P).astype(f16)

    in_maps = []
    for i in range(N_CORES):
        sl = slice(i * S_PER_CORE, (i + 1) * S_PER_CORE)
        att = np.empty((P, 2, S_PER_CORE * N2), dtype=f16)
        att[:, 0, :] = va[sl].reshape(S_PER_CORE * PPS, T * N2)
        att[:, 1, :] = ha[sl].reshape(S_PER_CORE * PPS, T * N2)
        att = np.ascontiguousarray(att.reshape(P, 2 * S_PER_CORE * N2))
        in_maps.append({
            "logits": lg[sl],
            "labels": lb[sl],
            "att": att,
            "poolmat": poolm,
        })
    return in_maps


def kernel(logits, labels, v_attention, h_attention):
    nc = _get_nc()
    in_maps = _make_in_maps(logits, labels, v_attention, h_attention)
    res = run_bass_kernel_spmd(nc, in_maps, core_ids=list(range(N_CORES)))
    stats_all = np.stack(
        [r["stats"].astype(np.float64) for r in res.results], axis=0)
    stats_act = np.stack(
        [r["stats_act"].astype(np.float64) for r in res.results], axis=0)
    return _host_combine(stats_all, stats_act)


# revision 29
# speedup vs baseline: 1.0625x; 1.0625x over previous
"""CovLoss (BCE + Dice + triple-Pearson) Trainium2 Bass kernel, v2.2.

Data parallel over batch: 32 samples -> 8 cores x 4 samples. Each core
streams fp16 logits/labels once, emits per-partition partial sums; host
combines in float64.

Engine plan (cost-model driven):
  - ACT: dummy Ln first (act-table load overlaps the first DMA), then
    Ln(p) / Ln(1-p)+accum per sample (engine floor ~15us), Square+accum
    for v2/h2, and two batched PSUM collapses (Copy+accum over bank rows;
    only rows 0/32/64 are meaningful, the rest is ignored garbage).
  - DVE: only ops with perf modes: tensor_scalar (4x) for masks/accums,
    tensor_tensor (2x) for the big products (D=lnp-lnq, y*D, y*M).
    scalar_tensor_tensor / tensor_tensor_reduce / custom DVE ops have NO
    perf modes (v1's mistake). Pool-finish kept on DVE (batched 2 samples
    per tensor_reduce over a 2-bank PSUM tile).
  - PE: row-pool matmuls (pooling), ones-reduces of y*D (16 matmuls into
    one accumulation row) and of y*M (per-sample rows at partition bases
    0/32/64 - the only legal matmul output bases).
  - GPSIMD can only memset/DMA (walrus rejects its tensor ops).
  - Queue discipline: each engine's emission order matches data readiness
    (in-order sequencers); poolfin after the products, l-moment tail
    interleaved into sample 3.
"""

import numpy as np

import concourse.bass as bass
import concourse.bacc as bacc
import concourse.tile as tile
from concourse import mybir
from concourse.bass_utils import run_bass_kernel_spmd

N_CORES = 8
N = 32
S_PER_CORE = N // N_CORES  # 4
H = W = 512
P = 128
T = H // P                 # 4 row blocks
FD = T * W                 # 2048 free elems per partition per sample
N2 = H // 4                # 128 pooled
K = N2 * N2
PPS = P // S_PER_CORE      # 32 partitions per sample in fat layout

F16 = mybir.dt.float16
F32 = mybir.dt.float32

# stats [128, 32] fp32 columns
C_M = 0      # cols 0..4: sum(M) (sample 0 split in two halves)
C_MYP = 5    # rows 0/32/64: sum(M*y) for samples 0..2 (PSUM collapse A)
C_BCP = 6    # row 32: sum(y*(lnp-lnq)) for samples 0..2 (PSUM collapse B)
C_BC3 = 7    # per-partition sum(y*D) sample 3 (DVE accum; PE is cold then)
C_MY3 = 8    # per-partition sum(M*y) sample 3 (DVE accum)
C_V, C_H, C_VH, C_V2, C_H2, C_VL, C_HL, C_VHL, C_L2, C_L = range(10, 20)
STATS_W = 32

ADD = mybir.AluOpType.add
MULT = mybir.AluOpType.mult
LN = mybir.ActivationFunctionType.Ln
SQUARE = mybir.ActivationFunctionType.Square
COPY = mybir.ActivationFunctionType.Copy


def _build_nc():
    nc = bacc.Bacc(trn_type="TRN2")

    d_logits = nc.dram_tensor("logits", [S_PER_CORE, P, FD], F16,
                              kind="ExternalInput")
    d_labels = nc.dram_tensor("labels", [S_PER_CORE, P, FD], F16,
                              kind="ExternalInput")
    d_att = nc.dram_tensor("att", [P, 2 * S_PER_CORE * N2], F16,
                           kind="ExternalInput")
    d_pool = nc.dram_tensor("poolmat", [P, T * P], F16, kind="ExternalInput")

    d_lpool = nc.dram_tensor("lpool", [S_PER_CORE, PPS, T, N2], F16,
                             kind="Internal")
    d_stats = nc.dram_tensor("stats", [P, STATS_W], F32,
                             kind="ExternalOutput")
    d_stats_act = nc.dram_tensor("stats_act", [P, S_PER_CORE + 1], F32,
                                 kind="ExternalOutput")

    with tile.TileContext(nc) as tc:
        with (
            tc.tile_pool(name="consts", bufs=1) as consts,
            tc.tile_pool(name="big", bufs=3) as big,
            tc.tile_pool(name="psum", bufs=2, space="PSUM") as psump,
            tc.tile_pool(name="psred", bufs=1, space="PSUM") as psred,
        ):
            stats = consts.tile([P, STATS_W], F32)
            stats_act = consts.tile([P, S_PER_CORE + 1], F32)
            attm = consts.tile([P, 2, S_PER_CORE * N2], F16)
            poolm = consts.tile([P, T, P], F16)
            lpool = consts.tile([P, S_PER_CORE, N2], F16)
            lfat = consts.tile([P, S_PER_CORE * N2], F16)
            vh = consts.tile([P, S_PER_CORE * N2], F16)
            ones = consts.tile([P, 1], F16)
            zeros65 = consts.tile([P, 65], F16)
            zerosw = consts.tile([P, W], F16)
            bias0 = consts.tile([P, 1], F16)
            bias1 = consts.tile([P, 1], F16)
            junkf = consts.tile([P, S_PER_CORE * N2], F16)
            junkv = consts.tile([P, S_PER_CORE * N2], F16)
            junkw = consts.tile([P, S_PER_CORE * N2], F16)
            cjunkA = consts.tile([65, W], F16)
            cjunkB = consts.tile([33, W], F16)

            vt = attm[:, 0, :]
            ht = attm[:, 1, :]

            nc.vector.memset(ones, 1.0)
            nc.gpsimd.memset(bias0, 0.0)
            nc.gpsimd.memset(bias1, 1.0)

            # dummy Ln: act-table load happens during the first input DMA
            nc.scalar.activation(out=bias1, in_=ones, func=LN, bias=bias0)
            nc.gpsimd.memset(bias1, 1.0)

            bankA = psred.tile([P, W], F32)   # rows 0/32/64: sum(M*y) s0..2
            bankB = psred.tile([P, W], F32)   # row 0: sum(M*y) s3; row 32: bce
            # zero the collapse windows so the Copy+accum reads no garbage
            nc.vector.memset(zeros65, 0.0)
            nc.gpsimd.memset(zerosw, 0.0)
            nc.tensor.matmul(bankA[0:65, :], lhsT=zeros65, rhs=zerosw,
                             start=True, stop=True, skip_group_check=True)
            nc.tensor.matmul(bankB[0:33, :], lhsT=zeros65[:, 0:33], rhs=zerosw,
                             start=True, stop=True, skip_group_check=True)
            # PE warmup: keep the tensor engine busy until the first pooling
            # matmuls so it reaches full pstate (cold PE runs 3.7x slower and
            # its latency cascades into the DVE pool-finish chain)
            for _ in range(14):
                nc.tensor.matmul(bankB[64:65, :], lhsT=zeros65[:, 0:1],
                                 rhs=zerosw, start=True, stop=True,
                                 skip_group_check=True)

            # input DMAs up front (SP in-order; p feeds ACT = critical)
            pts, yts = [], []
            for s in range(S_PER_CORE):
                p_tile = big.tile([P, FD], F16, tag=f"p{s}", name=f"p{s}")
                y_tile = big.tile([P, T, W], F16, tag=f"y{s}", name=f"y{s}")
                pts.append(p_tile)
                yts.append(y_tile)
            nc.sync.dma_start(out=pts[0][:, 0:FD // 2],
                              in_=d_logits[0][:, 0:FD // 2])
            nc.sync.dma_start(out=pts[0][:, FD // 2:FD],
                              in_=d_logits[0][:, FD // 2:FD])
            nc.sync.dma_start(
                out=yts[0], in_=d_labels[0].rearrange("p (t w) -> p t w", t=T))
            nc.sync.dma_start(
                out=attm, in_=d_att.rearrange("p (q f) -> p q f", q=2))
            nc.sync.dma_start(
                out=poolm, in_=d_pool.rearrange("p (t m) -> p t m", t=T))
            for s in (1, 2, 3):
                nc.sync.dma_start(out=pts[s], in_=d_logits[s])
                nc.sync.dma_start(
                    out=yts[s],
                    in_=d_labels[s].rearrange("p (t w) -> p t w", t=T))

            # PE: row-pool matmuls, interleaved with the reduces by readiness
            ps_pools = []
            for pair in range(2):
                ps_pools.append(psump.tile([P, 2 * W], F32, tag="pool",
                                           name=f"pspool{pair}"))

            def emit_pooling(s):
                half = ps_pools[s // 2][:, (s % 2) * W:(s % 2) * W + W]
                for t in range(T):
                    nc.tensor.matmul(
                        half, lhsT=poolm[:, t, :], rhs=yts[s][:, t, :],
                        start=(t == 0), stop=(t == T - 1),
                        skip_group_check=True)

            emit_pooling(0)
            emit_pooling(1)

            def poolfin(pair):
                with nc.allow_low_precision(reason="16-term pooled sums"):
                    nc.vector.tensor_reduce(
                        out=lpool.rearrange(
                            "p (u v) m -> p u v m", u=2)[:, pair],
                        in_=ps_pools[pair].rearrange(
                            "p (v g f) -> p v g f", v=2, f=4),
                        axis=mybir.AxisListType.X, op=ADD)

            for s in range(S_PER_CORE):
                pt, yt = pts[s], yts[s]
                ytf = yt.rearrange("p t w -> p (t w)")

                # mask: 4x tensor_scalar, accum -> sum(M) per sample
                # (sample 0 is split in halves to cut the startup latency;
                # the first half starts as soon as its DMA lands)
                dm = big.tile([P, 2, FD], F16, tag="dm")
                mt = dm[:, 1, :]
                if s == 0:
                    nc.vector.tensor_scalar(
                        out=mt[:, 0:FD // 2], in0=pt[:, 0:FD // 2],
                        scalar1=0.4, scalar2=None,
                        op0=mybir.AluOpType.is_gt, op1=ADD,
                        accum_out=stats[:, C_M:C_M + 1])
                    nc.vector.tensor_scalar(
                        out=mt[:, FD // 2:FD], in0=pt[:, FD // 2:FD],
                        scalar1=0.4, scalar2=None,
                        op0=mybir.AluOpType.is_gt, op1=ADD,
                        accum_out=stats[:, C_M + 1:C_M + 2])
                else:
                    nc.vector.tensor_scalar(
                        out=mt, in0=pt, scalar1=0.4, scalar2=None,
                        op0=mybir.AluOpType.is_gt, op1=ADD,
                        accum_out=stats[:, C_M + 1 + s:C_M + 2 + s])

                if s == 0:
                    # attention moments needing only v,h (fills DVE idle)
                    nc.vector.tensor_tensor(out=vh, in0=vt, in1=ht, op=MULT)
                    nc.vector.tensor_scalar(
                        out=junkf, in0=vh, scalar1=1.0, scalar2=None,
                        op0=MULT, op1=ADD, accum_out=stats[:, C_VH:C_VH + 1])
                    nc.vector.tensor_scalar(
                        out=junkf, in0=vt, scalar1=1.0, scalar2=None,
                        op0=MULT, op1=ADD, accum_out=stats[:, C_V:C_V + 1])
                    nc.vector.tensor_scalar(
                        out=junkf, in0=ht, scalar1=1.0, scalar2=None,
                        op0=MULT, op1=ADD, accum_out=stats[:, C_H:C_H + 1])

                # ACT: the two log passes (engine floor)
                lnp = big.tile([P, FD], F16, tag="lnp")
                lnq = big.tile([P, FD], F16, tag="lnq")
                if s == 0:
                    h = FD // 2
                    nc.scalar.activation(out=lnp[:, 0:h], in_=pt[:, 0:h],
                                         func=LN, bias=bias0)
                    nc.scalar.activation(
                        out=lnq[:, 0:h], in_=pt[:, 0:h], func=LN,
                        scale=-1.0, bias=bias1,
                        accum_out=stats_act[:, 0:1])
                    nc.scalar.activation(out=lnp[:, h:FD], in_=pt[:, h:FD],
                                         func=LN, bias=bias0)
                    nc.scalar.activation(
                        out=lnq[:, h:FD], in_=pt[:, h:FD], func=LN,
                        scale=-1.0, bias=bias1,
                        accum_out=stats_act[:, S_PER_CORE:S_PER_CORE + 1])
                else:
                    nc.scalar.activation(out=lnp, in_=pt, func=LN, bias=bias0)
                    nc.scalar.activation(
                        out=lnq, in_=pt, func=LN, scale=-1.0, bias=bias1,
                        accum_out=stats_act[:, s:s + 1])

                # DVE 2x products. My only needs the mask + labels, so the
                # scheduler can run it early while ACT produces the logs.
                nc.vector.tensor_tensor(
                    out=dm[:, 0, :], in0=lnp, in1=lnq,
                    op=mybir.AluOpType.subtract)
                if s == 2:
                    poolfin(1)
                    # bounce emitted before its readers (Tile deps follow
                    # emission order)
                    nc.sync.dma_start(
                        out=d_lpool.rearrange("s a b m -> (a b) s m"),
                        in_=lpool)
                    nc.sync.dma_start(
                        out=lfat,
                        in_=d_lpool.rearrange("s a b m -> (s a) (b m)"))
                yd = big.tile([P, T, W], F16, tag="yd")
                nc.vector.tensor_tensor(
                    out=yd.rearrange("p t w -> p (t w)"), in0=ytf,
                    in1=dm[:, 0, :], op=MULT)
                my = big.tile([P, T, W], F16, tag="my")
                nc.vector.tensor_tensor(
                    out=my.rearrange("p t w -> p (t w)"), in0=ytf, in1=mt,
                    op=MULT)

                if s == 1:
                    poolfin(0)

                # PE: ones-reduces for samples 0..2 (sample 3 runs on DVE;
                # PE is cold by then). y*D accumulates into bankB row 32;
                # y*M per sample into bankA rows 0/32/64.
                if s < 3:
                    for c in range(T):
                        nc.tensor.matmul(
                            bankB[32:33, :], lhsT=ones, rhs=yd[:, c, :],
                            start=(s == 0 and c == 0),
                            stop=(s == 2 and c == T - 1),
                            skip_group_check=True)
                    for c in range(T):
                        nc.tensor.matmul(
                            bankA[PPS * s:PPS * s + 1, :], lhsT=ones,
                            rhs=my[:, c, :],
                            start=(c == 0), stop=(c == T - 1),
                            skip_group_check=True)
                if s < 2:
                    emit_pooling(s + 2)
                if s == S_PER_CORE - 1:
                    # PE is cold by now; reduce sample 3 on DVE instead
                    ydf3 = yd.rearrange("p t w -> p (t w)")
                    nc.vector.tensor_scalar(
                        out=ydf3, in0=ydf3,
                        scalar1=1.0, scalar2=None, op0=MULT, op1=ADD,
                        accum_out=stats[:, C_BC3:C_BC3 + 1])
                    myf3 = my.rearrange("p t w -> p (t w)")
                    nc.vector.tensor_scalar(
                        out=myf3, in0=myf3,
                        scalar1=1.0, scalar2=None, op0=MULT, op1=ADD,
                        accum_out=stats[:, C_MY3:C_MY3 + 1])

                if s == S_PER_CORE - 1:
                    # l-moment tail on DVE (lfat ready via early bounce)
                    nc.vector.tensor_tensor(
                        out=junkf, in0=vt, in1=lfat, op=MULT)
                    nc.vector.tensor_scalar(
                        out=junkf, in0=junkf, scalar1=1.0, scalar2=None,
                        op0=MULT, op1=ADD,
                        accum_out=stats[:, C_VL:C_VL + 1])
                    nc.vector.tensor_tensor(
                        out=junkf, in0=ht, in1=lfat, op=MULT)
                    nc.vector.tensor_scalar(
                        out=junkf, in0=junkf, scalar1=1.0, scalar2=None,
                        op0=MULT, op1=ADD,
                        accum_out=stats[:, C_HL:C_HL + 1])
                    nc.vector.tensor_tensor(
                        out=junkf, in0=vh, in1=lfat, op=MULT)
                    nc.vector.tensor_scalar(
                        out=junkf, in0=junkf, scalar1=1.0, scalar2=None,
                        op0=MULT, op1=ADD,
                        accum_out=stats[:, C_VHL:C_VHL + 1])

            # ACT tail: v,h second moments (own junk tiles - no WAR with DVE)
            nc.scalar.activation(
                out=junkv, in_=vt, func=SQUARE, bias=bias0,
                accum_out=stats[:, C_V2:C_V2 + 1])
            nc.scalar.activation(
                out=junkw, in_=ht, func=SQUARE, bias=bias0,
                accum_out=stats[:, C_H2:C_H2 + 1])

            # batched PSUM collapses on ACT: per-partition row sums; host
            # reads only rows 0/32/64 (A) and 32 (B)
            nc.scalar.activation(
                out=cjunkA, in_=bankA[0:65, :], func=COPY,
                accum_out=stats[0:65, C_MYP:C_MYP + 1])
            nc.scalar.activation(
                out=cjunkB[0:1, :], in_=bankB[32:33, :], func=COPY,
                accum_out=stats[32:33, C_BCP:C_BCP + 1])
            # l moments that need no product ride the idle ACT tail
            nc.scalar.activation(
                out=junkv, in_=lfat, func=SQUARE, bias=bias0,
                accum_out=stats[:, C_L2:C_L2 + 1])
            nc.scalar.activation(
                out=junkw, in_=lfat, func=COPY,
                accum_out=stats[:, C_L:C_L + 1])

            nc.sync.dma_start(out=d_stats[:, :], in_=stats)
            nc.sync.dma_start(out=d_stats_act[:, :], in_=stats_act)

    nc.compile()
    return nc


_NC_CACHE = None


def _get_nc():
    global _NC_CACHE
    if _NC_CACHE is None:
        _NC_CACHE = _build_nc()
    return _NC_CACHE


def _host_combine(stats_all, stats_act):
    """stats_all: [N_CORES, P, STATS_W] float64 -> scalar loss (float32)."""
    smooth = 1.0
    bce_sum = 0.0
    dice_sum = 0.0
    cor_sum = 0.0
    for i in range(N_CORES):
        st = stats_all[i]
        bce_sum += (st[32, C_BCP] + st[:, C_BC3].sum()
                    + stats_act[i].sum())
        for s in range(S_PER_CORE):
            my = st[PPS * s, C_MYP] if s < 3 else st[:, C_MY3].sum()
            if s == 0:
                m_cnt = st[:, C_M].sum() + st[:, C_M + 1].sum()
            else:
                m_cnt = st[:, C_M + 1 + s].sum()
            part = slice(PPS * s, PPS * (s + 1))
            sv = st[part, C_V].sum()
            sh = st[part, C_H].sum()
            svh = st[part, C_VH].sum()
            sv2 = st[part, C_V2].sum()
            sh2 = st[part, C_H2].sum()
            svl = st[part, C_VL].sum()
            shl = st[part, C_HL].sum()
            svhl = st[part, C_VHL].sum()
            sl2 = st[part, C_L2].sum()
            sl = st[part, C_L].sum()

            dice_sum += 2.0 * (my + smooth) / (m_cnt + sl + smooth)

            mv, mh, ml = sv / K, sh / K, sl / K
            num = svhl - mv * shl - mh * svl - ml * svh + 2.0 * K * mv * mh * ml
            den = np.sqrt((sv2 - K * mv * mv) * (sh2 - K * mh * mh)
                          * (sl2 - K * ml * ml))
            cor_sum += num / den

    bceloss = -bce_sum / (N * H * W)
    diceloss = 1.0 - dice_sum / N
    cor_loss = -cor_sum / N
    return np.float32(0.2 * bceloss + 0.3 * diceloss + 0.5 * cor_loss)


def _make_in_maps(logits, labels, v_attention, h_attention):
    f16 = np.float16

    # clamp AFTER fp16 rounding so Ln(1-p) never sees exactly 1.0
    pmax = np.float16(1.0 - 2.0 ** -11)
    lg = np.minimum(np.asarray(logits, np.float32).astype(f16), pmax)
    # square layout: row r = 128*t + p  ->  partition p, free t*512+w
    lg = np.ascontiguousarray(
        lg.reshape(N, T, P, W).transpose(0, 2, 1, 3).reshape(N, P, FD))
    lb = np.asarray(labels, np.float32).astype(f16)
    lb = np.ascontiguousarray(
        lb.reshape(N, T, P, W).transpose(0, 2, 1, 3).reshape(N, P, FD))

    # fat attention layout: partition 32*s + a holds rows [4a, 4a+4)
    va = np.asarray(v_attention, np.float32).astype(f16).reshape(N, N2, N2)
    ha = np.asarray(h_attention, np.float32).astype(f16).reshape(N, N2, N2)

    # poolm[p, t, m] = 1 iff m == 32*t + p//4 (row-pool chunk t)
    poolm = np.zeros((P, T, P), dtype=np.float32)
    for t in range(T):
        poolm[np.arange(P), t, 32 * t + np.arange(P) // 4] = 1.0
    poolm = poolm.reshape(P, T * P).astype(f16)

    in_maps = []
    for i in range(N_CORES):
        sl = slice(i * S_PER_CORE, (i + 1) * S_PER_CORE)
        att = np.empty((P, 2, S_PER_CORE * N2), dtype=f16)
        att[:, 0, :] = va[sl].reshape(S_PER_CORE * PPS, T * N2)
        att[:, 1, :] = ha[sl].reshape(S_PER_CORE * PPS, T * N2)
        att = np.ascontiguousarray(att.reshape(P, 2 * S_PER_CORE * N2))
        in_maps.append({
            "logits": lg[sl],
            "labels": lb[sl],
            "att": att,
            "poolmat": poolm,
        })
    return in_maps


def kernel(logits, labels, v_attention, h_attention):
    nc = _get_nc()
    in_maps = _make_in_maps(logits, labels, v_attention, h_attention)
    res = run_bass_kernel_spmd(nc, in_maps, core_ids=list(range(N_CORES)))
    stats_all = np.stack(
        [r["stats"].astype(np.float64) for r in res.results], axis=0)
    stats_act = np.stack(
        [r["stats_act"].astype(np.float64) for r in res.results], axis=0)
    return _host_combine(stats_all, stats_act)


# revision 30
# speedup vs baseline: 1.0640x; 1.0014x over previous
"""CovLoss (BCE + Dice + triple-Pearson) Trainium2 Bass kernel, v2.2.

Data parallel over batch: 32 samples -> 8 cores x 4 samples. Each core
streams fp16 logits/labels once, emits per-partition partial sums; host
combines in float64.

Engine plan (cost-model driven):
  - ACT: dummy Ln first (act-table load overlaps the first DMA), then
    Ln(p) / Ln(1-p)+accum per sample (engine floor ~15us), Square+accum
    for v2/h2, and two batched PSUM collapses (Copy+accum over bank rows;
    only rows 0/32/64 are meaningful, the rest is ignored garbage).
  - DVE: only ops with perf modes: tensor_scalar (4x) for masks/accums,
    tensor_tensor (2x) for the big products (D=lnp-lnq, y*D, y*M).
    scalar_tensor_tensor / tensor_tensor_reduce / custom DVE ops have NO
    perf modes (v1's mistake). Pool-finish kept on DVE (batched 2 samples
    per tensor_reduce over a 2-bank PSUM tile).
  - PE: row-pool matmuls (pooling), ones-reduces of y*D (16 matmuls into
    one accumulation row) and of y*M (per-sample rows at partition bases
    0/32/64 - the only legal matmul output bases).
  - GPSIMD can only memset/DMA (walrus rejects its tensor ops).
  - Queue discipline: each engine's emission order matches data readiness
    (in-order sequencers); poolfin after the products, l-moment tail
    interleaved into sample 3.
"""

import numpy as np

import concourse.bass as bass
import concourse.bacc as bacc
import concourse.tile as tile
from concourse import mybir
from concourse.bass_utils import run_bass_kernel_spmd

N_CORES = 8
N = 32
S_PER_CORE = N // N_CORES  # 4
H = W = 512
P = 128
T = H // P                 # 4 row blocks
FD = T * W                 # 2048 free elems per partition per sample
N2 = H // 4                # 128 pooled
K = N2 * N2
PPS = P // S_PER_CORE      # 32 partitions per sample in fat layout

F16 = mybir.dt.float16
F32 = mybir.dt.float32

# stats [128, 32] fp32 columns
C_M = 0      # cols 0..4: sum(M) (sample 0 split in two halves)
C_MYP = 5    # rows 0/32/64: sum(M*y) for samples 0..2 (PSUM collapse A)
C_BCP = 6    # row 32: sum(y*(lnp-lnq)) for samples 0..2 (PSUM collapse B)
C_BC3 = 7    # per-partition sum(y*D) sample 3 (DVE accum; PE is cold then)
C_MY3 = 8    # per-partition sum(M*y) sample 3 (DVE accum)
C_V, C_H, C_VH, C_V2, C_H2, C_VL, C_HL, C_VHL, C_L2, C_L = range(10, 20)
STATS_W = 32

ADD = mybir.AluOpType.add
MULT = mybir.AluOpType.mult
LN = mybir.ActivationFunctionType.Ln
SQUARE = mybir.ActivationFunctionType.Square
COPY = mybir.ActivationFunctionType.Copy


def _build_nc():
    nc = bacc.Bacc(trn_type="TRN2")

    d_logits = nc.dram_tensor("logits", [S_PER_CORE, P, FD], F16,
                              kind="ExternalInput")
    d_labels = nc.dram_tensor("labels", [S_PER_CORE, P, FD], F16,
                              kind="ExternalInput")
    d_att = nc.dram_tensor("att", [P, 2 * S_PER_CORE * N2], F16,
                           kind="ExternalInput")
    d_pool = nc.dram_tensor("poolmat", [P, T * P], F16, kind="ExternalInput")

    d_lpool = nc.dram_tensor("lpool", [S_PER_CORE, PPS, T, N2], F16,
                             kind="Internal")
    d_stats = nc.dram_tensor("stats", [P, STATS_W], F32,
                             kind="ExternalOutput")
    d_stats_act = nc.dram_tensor("stats_act", [P, S_PER_CORE + 1], F32,
                                 kind="ExternalOutput")

    with tile.TileContext(nc) as tc:
        with (
            tc.tile_pool(name="consts", bufs=1) as consts,
            tc.tile_pool(name="big", bufs=3) as big,
            tc.tile_pool(name="psum", bufs=2, space="PSUM") as psump,
            tc.tile_pool(name="psred", bufs=1, space="PSUM") as psred,
        ):
            stats = consts.tile([P, STATS_W], F32)
            stats_act = consts.tile([P, S_PER_CORE + 1], F32)
            attm = consts.tile([P, 2, S_PER_CORE * N2], F16)
            poolm = consts.tile([P, T, P], F16)
            lpool = consts.tile([P, S_PER_CORE, N2], F16)
            lfat = consts.tile([P, S_PER_CORE * N2], F16)
            vh = consts.tile([P, S_PER_CORE * N2], F16)
            ones = consts.tile([P, 1], F16)
            gones = consts.tile([P, S_PER_CORE], F16)
            zeros65 = consts.tile([P, 65], F16)
            zerosw = consts.tile([P, W], F16)
            bias0 = consts.tile([P, 1], F16)
            bias1 = consts.tile([P, 1], F16)
            junkf = consts.tile([P, S_PER_CORE * N2], F16)
            junkv = consts.tile([P, S_PER_CORE * N2], F16)
            junkw = consts.tile([P, S_PER_CORE * N2], F16)
            cjunkA = consts.tile([65, W], F16)
            cjunkB = consts.tile([68, W], F16)

            vt = attm[:, 0, :]
            ht = attm[:, 1, :]

            nc.vector.memset(ones, 1.0)
            nc.gpsimd.memset(bias0, 0.0)
            nc.gpsimd.memset(bias1, 1.0)
            nc.gpsimd.memset(gones, 0.0)
            for g in range(S_PER_CORE):
                nc.gpsimd.memset(gones[PPS * g:PPS * (g + 1), g:g + 1], 1.0)

            # dummy Ln: act-table load happens during the first input DMA
            nc.scalar.activation(out=bias1, in_=ones, func=LN, bias=bias0)
            nc.gpsimd.memset(bias1, 1.0)

            bankA = psred.tile([P, W], F32)   # rows 0/32/64: sum(M*y) s0..2
            bankB = psred.tile([P, W], F32)   # row 0: sum(M*y) s3; row 32: bce
            # zero the collapse windows so the Copy+accum reads no garbage
            nc.vector.memset(zeros65, 0.0)
            nc.gpsimd.memset(zerosw, 0.0)
            nc.tensor.matmul(bankA[0:65, :], lhsT=zeros65, rhs=zerosw,
                             start=True, stop=True, skip_group_check=True)
            nc.tensor.matmul(bankB[0:65, :], lhsT=zeros65, rhs=zerosw,
                             start=True, stop=True, skip_group_check=True)
            # PE warmup: keep the tensor engine busy until the first pooling
            # matmuls so it reaches full pstate (cold PE runs 3.7x slower and
            # its latency cascades into the DVE pool-finish chain). Writes
            # bankA row 64, which my_2's start=True reduce resets later.
            for _ in range(14):
                nc.tensor.matmul(bankA[64:65, :], lhsT=zeros65[:, 0:1],
                                 rhs=zerosw, start=True, stop=True,
                                 skip_group_check=True)

            # input DMAs up front (SP in-order; p feeds ACT = critical)
            pts, yts = [], []
            for s in range(S_PER_CORE):
                p_tile = big.tile([P, FD], F16, tag=f"p{s}", name=f"p{s}")
                y_tile = big.tile([P, T, W], F16, tag=f"y{s}", name=f"y{s}")
                pts.append(p_tile)
                yts.append(y_tile)
            nc.sync.dma_start(out=pts[0][:, 0:FD // 2],
                              in_=d_logits[0][:, 0:FD // 2])
            nc.sync.dma_start(out=pts[0][:, FD // 2:FD],
                              in_=d_logits[0][:, FD // 2:FD])
            nc.sync.dma_start(
                out=yts[0], in_=d_labels[0].rearrange("p (t w) -> p t w", t=T))
            nc.sync.dma_start(
                out=attm, in_=d_att.rearrange("p (q f) -> p q f", q=2))
            nc.sync.dma_start(
                out=poolm, in_=d_pool.rearrange("p (t m) -> p t m", t=T))
            for s in (1, 2, 3):
                nc.sync.dma_start(out=pts[s], in_=d_logits[s])
                nc.sync.dma_start(
                    out=yts[s],
                    in_=d_labels[s].rearrange("p (t w) -> p t w", t=T))

            # PE: row-pool matmuls, interleaved with the reduces by readiness
            ps_pools = []
            for pair in range(2):
                ps_pools.append(psump.tile([P, 2 * W], F32, tag="pool",
                                           name=f"pspool{pair}"))

            def emit_pooling(s):
                half = ps_pools[s // 2][:, (s % 2) * W:(s % 2) * W + W]
                for t in range(T):
                    nc.tensor.matmul(
                        half, lhsT=poolm[:, t, :], rhs=yts[s][:, t, :],
                        start=(t == 0), stop=(t == T - 1),
                        skip_group_check=True)

            emit_pooling(0)
            emit_pooling(1)

            def poolfin(pair):
                with nc.allow_low_precision(reason="16-term pooled sums"):
                    nc.vector.tensor_reduce(
                        out=lpool.rearrange(
                            "p (u v) m -> p u v m", u=2)[:, pair],
                        in_=ps_pools[pair].rearrange(
                            "p (v g f) -> p v g f", v=2, f=4),
                        axis=mybir.AxisListType.X, op=ADD)

            for s in range(S_PER_CORE):
                pt, yt = pts[s], yts[s]
                ytf = yt.rearrange("p t w -> p (t w)")

                # mask: 4x tensor_scalar, accum -> sum(M) per sample
                # (sample 0 is split in halves to cut the startup latency;
                # the first half starts as soon as its DMA lands)
                dm = big.tile([P, 2, FD], F16, tag="dm")
                mt = dm[:, 1, :]
                if s == 0:
                    nc.vector.tensor_scalar(
                        out=mt[:, 0:FD // 2], in0=pt[:, 0:FD // 2],
                        scalar1=0.4, scalar2=None,
                        op0=mybir.AluOpType.is_gt, op1=ADD,
                        accum_out=stats[:, C_M:C_M + 1])
                    nc.vector.tensor_scalar(
                        out=mt[:, FD // 2:FD], in0=pt[:, FD // 2:FD],
                        scalar1=0.4, scalar2=None,
                        op0=mybir.AluOpType.is_gt, op1=ADD,
                        accum_out=stats[:, C_M + 1:C_M + 2])
                else:
                    nc.vector.tensor_scalar(
                        out=mt, in0=pt, scalar1=0.4, scalar2=None,
                        op0=mybir.AluOpType.is_gt, op1=ADD,
                        accum_out=stats[:, C_M + 1 + s:C_M + 2 + s])

                if s == 0:
                    # attention moments needing only v,h (fills DVE idle);
                    # Sv/Sh per sample ride PE group-ones reduces into
                    # bankB rows 0-3 / 64-67 (collapse B reads them out)
                    nc.vector.tensor_tensor(out=vh, in0=vt, in1=ht, op=MULT)
                    nc.vector.tensor_scalar(
                        out=junkf, in0=vh, scalar1=1.0, scalar2=None,
                        op0=MULT, op1=ADD, accum_out=stats[:, C_VH:C_VH + 1])
                    nc.tensor.matmul(bankB[0:4, :], lhsT=gones, rhs=vt,
                                     start=True, stop=True,
                                     skip_group_check=True)
                    nc.tensor.matmul(bankB[64:68, :], lhsT=gones, rhs=ht,
                                     start=True, stop=True,
                                     skip_group_check=True)

                # ACT: the two log passes (engine floor)
                lnp = big.tile([P, FD], F16, tag="lnp")
                lnq = big.tile([P, FD], F16, tag="lnq")
                if s == 0:
                    h = FD // 2
                    nc.scalar.activation(out=lnp[:, 0:h], in_=pt[:, 0:h],
                                         func=LN, bias=bias0)
                    nc.scalar.activation(
                        out=lnq[:, 0:h], in_=pt[:, 0:h], func=LN,
                        scale=-1.0, bias=bias1,
                        accum_out=stats_act[:, 0:1])
                    nc.scalar.activation(out=lnp[:, h:FD], in_=pt[:, h:FD],
                                         func=LN, bias=bias0)
                    nc.scalar.activation(
                        out=lnq[:, h:FD], in_=pt[:, h:FD], func=LN,
                        scale=-1.0, bias=bias1,
                        accum_out=stats_act[:, S_PER_CORE:S_PER_CORE + 1])
                else:
                    nc.scalar.activation(out=lnp, in_=pt, func=LN, bias=bias0)
                    nc.scalar.activation(
                        out=lnq, in_=pt, func=LN, scale=-1.0, bias=bias1,
                        accum_out=stats_act[:, s:s + 1])

                # DVE 2x products. My only needs the mask + labels, so the
                # scheduler can run it early while ACT produces the logs.
                nc.vector.tensor_tensor(
                    out=dm[:, 0, :], in0=lnp, in1=lnq,
                    op=mybir.AluOpType.subtract)
                if s == 2:
                    poolfin(1)
                    # bounce emitted before its readers (Tile deps follow
                    # emission order)
                    nc.sync.dma_start(
                        out=d_lpool.rearrange("s a b m -> (a b) s m"),
                        in_=lpool)
                    nc.sync.dma_start(
                        out=lfat,
                        in_=d_lpool.rearrange("s a b m -> (s a) (b m)"))
                yd = big.tile([P, T, W], F16, tag="yd")
                nc.vector.tensor_tensor(
                    out=yd.rearrange("p t w -> p (t w)"), in0=ytf,
                    in1=dm[:, 0, :], op=MULT)
                my = big.tile([P, T, W], F16, tag="my")
                nc.vector.tensor_tensor(
                    out=my.rearrange("p t w -> p (t w)"), in0=ytf, in1=mt,
                    op=MULT)

                if s == 1:
                    poolfin(0)

                # PE: ones-reduces for samples 0..2 (sample 3 runs on DVE;
                # PE is cold by then). y*D accumulates into bankB row 32;
                # y*M per sample into bankA rows 0/32/64.
                if s < 3:
                    for c in range(T):
                        nc.tensor.matmul(
                            bankB[32:33, :], lhsT=ones, rhs=yd[:, c, :],
                            start=(s == 0 and c == 0),
                            stop=(s == 2 and c == T - 1),
                            skip_group_check=True)
                    for c in range(T):
                        nc.tensor.matmul(
                            bankA[PPS * s:PPS * s + 1, :], lhsT=ones,
                            rhs=my[:, c, :],
                            start=(c == 0), stop=(c == T - 1),
                            skip_group_check=True)
                if s < 2:
                    emit_pooling(s + 2)
                if s == S_PER_CORE - 1:
                    # PE is cold by now; reduce sample 3 on DVE instead
                    ydf3 = yd.rearrange("p t w -> p (t w)")
                    nc.vector.tensor_scalar(
                        out=ydf3, in0=ydf3,
                        scalar1=1.0, scalar2=None, op0=MULT, op1=ADD,
                        accum_out=stats[:, C_BC3:C_BC3 + 1])
                    myf3 = my.rearrange("p t w -> p (t w)")
                    nc.vector.tensor_scalar(
                        out=myf3, in0=myf3,
                        scalar1=1.0, scalar2=None, op0=MULT, op1=ADD,
                        accum_out=stats[:, C_MY3:C_MY3 + 1])

                if s == S_PER_CORE - 1:
                    # l-moment tail on DVE (lfat ready via early bounce)
                    nc.vector.tensor_tensor(
                        out=junkf, in0=vt, in1=lfat, op=MULT)
                    nc.vector.tensor_scalar(
                        out=junkf, in0=junkf, scalar1=1.0, scalar2=None,
                        op0=MULT, op1=ADD,
                        accum_out=stats[:, C_VL:C_VL + 1])
                    nc.vector.tensor_tensor(
                        out=junkf, in0=ht, in1=lfat, op=MULT)
                    nc.vector.tensor_scalar(
                        out=junkf, in0=junkf, scalar1=1.0, scalar2=None,
                        op0=MULT, op1=ADD,
                        accum_out=stats[:, C_HL:C_HL + 1])
                    nc.vector.tensor_tensor(
                        out=junkf, in0=vh, in1=lfat, op=MULT)
                    nc.vector.tensor_scalar(
                        out=junkf, in0=junkf, scalar1=1.0, scalar2=None,
                        op0=MULT, op1=ADD,
                        accum_out=stats[:, C_VHL:C_VHL + 1])

            # ACT tail: v,h second moments (own junk tiles - no WAR with DVE)
            nc.scalar.activation(
                out=junkv, in_=vt, func=SQUARE, bias=bias0,
                accum_out=stats[:, C_V2:C_V2 + 1])
            nc.scalar.activation(
                out=junkw, in_=ht, func=SQUARE, bias=bias0,
                accum_out=stats[:, C_H2:C_H2 + 1])

            # batched PSUM collapses on ACT: per-partition row sums; host
            # reads only rows 0/32/64 (A) and 32 (B)
            nc.scalar.activation(
                out=cjunkA, in_=bankA[0:65, :], func=COPY,
                accum_out=stats[0:65, C_MYP:C_MYP + 1])
            nc.scalar.activation(
                out=cjunkB, in_=bankB[0:68, :], func=COPY,
                accum_out=stats[0:68, C_BCP:C_BCP + 1])
            # l moments that need no product ride the idle ACT tail
            nc.scalar.activation(
                out=junkv, in_=lfat, func=SQUARE, bias=bias0,
                accum_out=stats[:, C_L2:C_L2 + 1])
            nc.scalar.activation(
                out=junkw, in_=lfat, func=COPY,
                accum_out=stats[:, C_L:C_L + 1])

            nc.sync.dma_start(out=d_stats[:, :], in_=stats)
            nc.sync.dma_start(out=d_stats_act[:, :], in_=stats_act)

    nc.compile()
    return nc


_NC_CACHE = None


def _get_nc():
    global _NC_CACHE
    if _NC_CACHE is None:
        _NC_CACHE = _build_nc()
    return _NC_CACHE


def _host_combine(stats_all, stats_act):
    """stats_all: [N_CORES, P, STATS_W] float64 -> scalar loss (float32)."""
    smooth = 1.0
    bce_sum = 0.0
    dice_sum = 0.0
    cor_sum = 0.0
    for i in range(N_CORES):
        st = stats_all[i]
        bce_sum += (st[32, C_BCP] + st[:, C_BC3].sum()
                    + stats_act[i].sum())
        for s in range(S_PER_CORE):
            my = st[PPS * s, C_MYP] if s < 3 else st[:, C_MY3].sum()
            if s == 0:
                m_cnt = st[:, C_M].sum() + st[:, C_M + 1].sum()
            else:
                m_cnt = st[:, C_M + 1 + s].sum()
            part = slice(PPS * s, PPS * (s + 1))
            sv = st[s, C_BCP]
            sh = st[64 + s, C_BCP]
            svh = st[part, C_VH].sum()
            sv2 = st[part, C_V2].sum()
            sh2 = st[part, C_H2].sum()
            svl = st[part, C_VL].sum()
            shl = st[part, C_HL].sum()
            svhl = st[part, C_VHL].sum()
            sl2 = st[part, C_L2].sum()
            sl = st[part, C_L].sum()

            dice_sum += 2.0 * (my + smooth) / (m_cnt + sl + smooth)

            mv, mh, ml = sv / K, sh / K, sl / K
            num = svhl - mv * shl - mh * svl - ml * svh + 2.0 * K * mv * mh * ml
            den = np.sqrt((sv2 - K * mv * mv) * (sh2 - K * mh * mh)
                          * (sl2 - K * ml * ml))
            cor_sum += num / den

    bceloss = -bce_sum / (N * H * W)
    diceloss = 1.0 - dice_sum / N
    cor_loss = -cor_sum / N
    return np.float32(0.2 * bceloss + 0.3 * diceloss + 0.5 * cor_loss)


def _make_in_maps(logits, labels, v_attention, h_attention):
    f16 = np.float16

    # clamp AFTER fp16 rounding so Ln(1-p) never sees exactly 1.0
    pmax = np.float16(1.0 - 2.0 ** -11)
    lg = np.minimum(np.asarray(logits, np.float32).astype(f16), pmax)
    # square layout: row r = 128*t + p  ->  partition p, free t*512+w
    lg = np.ascontiguousarray(
        lg.reshape(N, T, P, W).transpose(0, 2, 1, 3).reshape(N, P, FD))
    lb = np.asarray(labels, np.float32).astype(f16)
    lb = np.ascontiguousarray(
        lb.reshape(N, T, P, W).transpose(0, 2, 1, 3).reshape(N, P, FD))

    # fat attention layout: partition 32*s + a holds rows [4a, 4a+4)
    va = np.asarray(v_attention, np.float32).astype(f16).reshape(N, N2, N2)
    ha = np.asarray(h_attention, np.float32).astype(f16).reshape(N, N2, N2)

    # poolm[p, t, m] = 1 iff m == 32*t + p//4 (row-pool chunk t)
    poolm = np.zeros((P, T, P), dtype=np.float32)
    for t in range(T):
        poolm[np.arange(P), t, 32 * t + np.arange(P) // 4] = 1.0
    poolm = poolm.reshape(P, T * P).astype(f16)

    in_maps = []
    for i in range(N_CORES):
        sl = slice(i * S_PER_CORE, (i + 1) * S_PER_CORE)
        att = np.empty((P, 2, S_PER_CORE * N2), dtype=f16)
        att[:, 0, :] = va[sl].reshape(S_PER_CORE * PPS, T * N2)
        att[:, 1, :] = ha[sl].reshape(S_PER_CORE * PPS, T * N2)
        att = np.ascontiguousarray(att.reshape(P, 2 * S_PER_CORE * N2))
        in_maps.append({
            "logits": lg[sl],
            "labels": lb[sl],
            "att": att,
            "poolmat": poolm,
        })
    return in_maps


def kernel(logits, labels, v_attention, h_attention):
    nc = _get_nc()
    in_maps = _make_in_maps(logits, labels, v_attention, h_attention)
    res = run_bass_kernel_spmd(nc, in_maps, core_ids=list(range(N_CORES)))
    stats_all = np.stack(
        [r["stats"].astype(np.float64) for r in res.results], axis=0)
    stats_act = np.stack(
        [r["stats_act"].astype(np.float64) for r in res.results], axis=0)
    return _host_combine(stats_all, stats_act)
